# revision 2
# baseline (speedup 1.0000x reference)
"""DWHT (buggy in-place Walsh-Hadamard channel transform + channel shuffle) on 8 trn2 cores.

The whole nn.Module is a fixed linear map on the channel axis:
    y[b, :, h, w] = T @ x[b, :, h, w]
with T a (512, 256) matrix of small integers (|T| <= 13, exactly representable
in bf16).  Batch 64 is sharded 8-ways (data parallel, 8 samples/core); each
core runs a tiled PE matmul: for every sample, y_s (512,784) = T @ x_s (256,784).

Precision strategy (variant "bf16_io", default): the checker gate is 2e-2
relative error, so 16-bit I/O is comfortably inside tolerance.  The host
wrapper rounds x to bf16 (input rel err ~4e-4 RMS), the device computes
T@x with an exact-bf16 T accumulating in fp32 PSUM, and the output is
evicted PSUM->SBUF with a bf16 cast and DMA'd out as bf16; the host
upcasts to fp32.  Measured rel err ~9e-4.  This halves HBM traffic vs
fp32 I/O: per core 3.2MB in + 6.4MB out (+0.26MB weights) ~= 9.9MB.

Schedule: input DMAs and half the output DMAs ride the SP (sync) queue,
the other output DMAs ride the Pool (gpsimd) queue; PSUM->SBUF bf16
eviction alternates DVE/ACT; PE does 16 bf16 matmuls per sample (2
k-chunks x 4 m-tiles x 2 spatial chunks of 392 <= one PSUM bank).
"""

import os
import sys

import numpy as np

for _p in ("/opt/trn_rl_repo", "/root/.axon_site/_ro/trn_rl_repo"):
    if os.path.isdir(_p) and _p not in sys.path:
        sys.path.append(_p)

B, C_IN, C_OUT, HH, WW = 64, 256, 512, 28, 28
S = HH * WW  # 784
N_CORES = 8
BS = B // N_CORES  # 8 samples per core
N_PASSES, GROUPS = 8, 8

VARIANT = os.environ.get("DWHT_VARIANT", "bf16_io")

# spatial split per PSUM bank (each chunk <= 512 fp32 = one bank)
N_CHUNKS = ((0, 392), (392, 392))


def _dwht_T() -> np.ndarray:
    """Build the (512, 256) transform matrix by running the reference
    butterfly (including its partial-update in-place semantics) on identity."""
    x = np.zeros((C_OUT, C_IN), np.float64)
    x[:C_IN] = np.eye(C_IN)
    half = C_OUT // 2
    for _ in range(N_PASSES):
        top = x[::2] + x[1::2]
        x = x.copy()
        x[:half] = top
        bottom = x[::2] - x[1::2]
        x[half:] = bottom
    # channel shuffle with groups=8
    x = x.reshape(GROUPS, C_OUT // GROUPS, C_IN).transpose(1, 0, 2).reshape(C_OUT, C_IN)
    return x


def _build_bf16(reps=1):
    """bf16-I/O variant: x, tt, y all bf16 in DRAM; fp32 PSUM accumulate."""
    import concourse.mybir as mybir
    from concourse import bacc
    from concourse.tile import TileContext

    f32 = mybir.dt.float32
    bf16 = mybir.dt.bfloat16

    in_q = os.environ.get("DWHT_IN_Q", "sync")
    out_qs = os.environ.get("DWHT_OUT_QS", "sync,gpsimd").split(",")
    copy_qs = os.environ.get("DWHT_COPY_QS", "vector,scalar").split(",")

    nc = bacc.Bacc(None, target_bir_lowering=False)
    x = nc.dram_tensor("x", (BS, C_IN, S), bf16, kind="ExternalInput")
    tt = nc.dram_tensor("tt", (C_IN, C_OUT), bf16, kind="ExternalInput")
    y = nc.dram_tensor("y", (BS, C_OUT, S), bf16, kind="ExternalOutput")

    def eng(name):
        return getattr(nc, name)

    def copy_op(name, dst, src):
        if name == "vector":
            nc.vector.tensor_copy(dst, src)
        else:
            eng(name).copy(dst, src)

    with TileContext(nc) as tc:
        with (
            tc.tile_pool(name="w", bufs=1) as wp,
            tc.tile_pool(name="io", bufs=3) as io,
            tc.tile_pool(name="ps", bufs=8, space="PSUM") as pp,
        ):
            tts = []
            for k in range(2):
                t = wp.tile([128, C_OUT], bf16, tag=f"tt{k}")
                nc.sync.dma_start(out=t[:], in_=tt[k * 128 : (k + 1) * 128, :])
                tts.append(t)

            sample_seq = [s for _ in range(reps) for s in range(BS)]
            ndma = 0
            for si, s in enumerate(sample_seq):
                last_sample = si == len(sample_seq) - 1
                xsk = []
                for k in range(2):
                    xs = io.tile([128, S], bf16, tag="xs", bufs=8)
                    eng(in_q).dma_start(
                        out=xs[:], in_=x[s, k * 128 : (k + 1) * 128, :]
                    )
                    xsk.append(xs)

                for m in range(C_OUT // 128):
                    msl = slice(m * 128, (m + 1) * 128)
                    ysm = io.tile([128, S], bf16, tag="ysm", bufs=6, name="ysm")
                    for ni, (n0, nsz) in enumerate(N_CHUNKS):
                        nsl = slice(n0, n0 + nsz)
                        ps = pp.tile([128, nsz], f32, tag="ps")
                        for ki in range(2):
                            nc.tensor.matmul(
                                ps[:],
                                tts[ki][:, msl],
                                xsk[ki][:, nsl],
                                start=(ki == 0),
                                stop=(ki == 1),
                            )
                        cq = copy_qs[(m * len(N_CHUNKS) + ni) % len(copy_qs)]
                        copy_op(cq, ysm[:, nsl], ps[:])
                    oq = out_qs[ndma % len(out_qs)]
                    ndma += 1
                    eng(oq).dma_start(out=y[s, msl, :], in_=ysm[:])

    nc.compile()
    return nc


_cache = {}


def _get_nc(variant, reps=1):
    key = (variant, reps, os.environ.get("DWHT_IN_Q"), os.environ.get("DWHT_OUT_QS"),
           os.environ.get("DWHT_COPY_QS"))
    if key not in _cache:
        if variant != "bf16_io":
            raise ValueError(variant)
        _cache[key] = _build_bf16(reps)
    return _cache[key]


def _in_maps(x_np, variant):
    import ml_dtypes

    T = _dwht_T()
    ttT = np.ascontiguousarray(T.T)  # (256, 512), lhsT layout
    tt_np = ttT.astype(ml_dtypes.bfloat16)
    x_bf = x_np.astype(ml_dtypes.bfloat16)
    return [
        {"x": x_bf[i * BS : (i + 1) * BS], "tt": tt_np} for i in range(N_CORES)
    ]


def _run(x_np, variant=None, trace=False, reps=1):
    from concourse.bass_utils import run_bass_kernel_spmd

    variant = variant or VARIANT
    nc = _get_nc(variant, reps)
    res = run_bass_kernel_spmd(
        nc, _in_maps(x_np, variant), list(range(N_CORES)), trace=trace
    )
    y = (
        np.stack([np.asarray(r["y"], np.float32) for r in res.results])
        .reshape(B, C_OUT, HH, WW)
    )
    return y, res


def kernel(x: np.ndarray) -> np.ndarray:
    x_np = np.ascontiguousarray(np.asarray(x), dtype=np.float32).reshape(B, C_IN, S)
    y, _ = _run(x_np)
    return y
